# revision 63
# baseline (speedup 1.0000x reference)
"""FeatureProcessingBlock Trainium kernel (bf16 pipeline, v4).

out = sum_t einsum('bcphqw,twW,thH,tcC->bCpHqW', x.reshape(B,C,P,64,Q,64), Ws, Hs, Cs)

Sharding: 8 cores = (B=4) x (H-halves=2); each core gets x[b, :, ph*256:(ph+1)*256, :]
a [C=48, 256, 512] slab = 4 p-blocks x 4 double-windows (dw = two adjacent
64x64 windows in a 128-wide w-slab).

Per-core pipeline, all matmul operands bf16 (PSUM accumulates f32):
  c-stage  (data-stationary, h-pair packed): lhsT = x[(par,c)96, wp128] chunk,
            rhs = [cstk_lo | cstk_hi] [96, 288] (zero-padded halves per parity)
            -> PSUM [128 wp, (par2, t3, j24, cs2)]
            -> Scalar drain/cast to Ybuf [128 (win,w), (t3, j24, h64, cs2)]
  wT-stage (fused w-matmul + transpose; data-stationary): lhsT = Ybuf (t,j)-chunk
            [128 (win,w), 128 (h,cs)], rhs = blkdiag(Ws_t, Ws_t)
            -> PSUM [128 (h,cs), (win, W')] -- already transposed for the h-stage
            -> Vector/Scalar drain/cast to ZT[t] bf16
  h-stage  (t-sum in PSUM): lhsT = Hblk2[t] (rows 2h+cs, cols (cs,H')),
            rhs = ZT[t] chunks, 3-matmul accumulation
            -> O [128 (cs, H'), (j8, W')] -> Vector drain f32 -> DMA out
"""

import sys
import types

import numpy as np


def _ensure_ntff_hook_module():
    """concourse.bass_utils imports antenv.axon_hooks when BASS_TRACE is set;
    provide a fallback module (wired to the ctypes NTFF hook when available)
    so tracing degrades gracefully instead of crashing."""
    try:
        import antenv.axon_hooks  # noqa: F401

        return
    except ImportError:
        pass
    mod = types.ModuleType("antenv.axon_hooks")
    mod._hook = None
    mod.set_axon_ntff_profile_hook = lambda h: setattr(mod, "_hook", h)
    mod.get_axon_ntff_profile_hook = lambda: mod._hook
    sys.modules["antenv.axon_hooks"] = mod
    try:
        from trn_agent_boot.trn_boot import _ntff_profile_via_ctypes

        mod._hook = _ntff_profile_via_ctypes("/opt/axon/libaxon_pjrt.so")
    except Exception:
        pass


_ensure_ntff_hook_module()

B, C, H, W = 4, 48, 512, 512
T, WS = 3, 64
NCORES = 8
PH = H // 2    # 256 rows per core
NP = PH // 64  # 4 p-blocks

LAST_EXEC_NS = None
_CACHE = {}


def _build():
    import concourse.bacc as bacc
    import concourse.mybir as mybir
    from concourse.bass import MemorySpace
    from concourse.tile import TileContext

    F32 = mybir.dt.float32
    BF16 = mybir.dt.bfloat16

    nc = bacc.Bacc("TRN2", target_bir_lowering=False, debug=False, num_devices=NCORES)
    x = nc.dram_tensor("x", [C, PH, W], BF16, kind="ExternalInput")
    cstk = nc.dram_tensor("cstk", [96, 2 * T * C], BF16, kind="ExternalInput")
    wblk = nc.dram_tensor("wblk", [T, 128, 128], BF16, kind="ExternalInput")
    hblk = nc.dram_tensor("hblk", [T, 128, 128], BF16, kind="ExternalInput")
    out = nc.dram_tensor("out", [C, PH, W], BF16, kind="ExternalOutput")

    with TileContext(nc) as tc:
        with (
            tc.tile_pool(name="consts", bufs=1) as consts,
            tc.tile_pool(name="xin", bufs=3) as xin,
            tc.tile_pool(name="xfirst", bufs=1) as xfirst,
            tc.tile_pool(name="ybuf", bufs=2) as ypool,
            tc.tile_pool(name="ztbuf", bufs=2) as ztpool,
            tc.tile_pool(name="obuf", bufs=2) as opool,
            tc.tile_pool(name="cps", bufs=3, space=MemorySpace.PSUM) as cps,
            tc.tile_pool(name="tps", bufs=3, space=MemorySpace.PSUM) as tps,
            tc.tile_pool(name="ops", bufs=2, space=MemorySpace.PSUM) as ops,
        ):
            cstk_sb = consts.tile([96, T, 24, 2, 2], BF16)
            nc.sync.dma_start(
                out=cstk_sb,
                in_=cstk[:, :].rearrange(
                    "k (t j par s) -> k t j par s", t=T, j=24, par=2
                ),
            )
            wblk_sb = consts.tile([128, T, 128], BF16)
            hblk_sb = consts.tile([128, T, 128], BF16)
            wh_loaded = []

            def load_wh():
                nc.sync.dma_start(
                    out=wblk_sb, in_=wblk[:, :, :].rearrange("t k m -> k t m")
                )
                nc.sync.dma_start(
                    out=hblk_sb, in_=hblk[:, :, :].rearrange("t k m -> k t m")
                )
                wh_loaded.append(True)

            obs = {}

            def emit_c_stage(xt, yb, d):
                wp0 = 128 * (d % 2)
                chunks = []
                for hh in range(32):
                    def c_chunk(hh=hh, yb=yb, wp0=wp0):
                        xq, r = xt[hh]
                        cp = cps.tile([128, T, 24, 2, 2], F32, tag="c")
                        nc.tensor.matmul(
                            cp,
                            lhsT=xq[:, r, wp0 : wp0 + 128],
                            rhs=cstk_sb,
                            start=True,
                            stop=True,
                        )
                        dst = yb[:, :, :, 2 * hh : 2 * hh + 2, :]
                        if hh % 2 == 1:
                            nc.vector.tensor_copy(out=dst, in_=cp)
                        else:
                            nc.scalar.copy(out=dst, in_=cp)
                    chunks.append(c_chunk)
                return chunks

            def make_groups(p, d, yb, last=False):
                """wT + h + DMA emission callbacks for one block, ordered so
                each cc's h-stage depends only on its own 6 wT sub-tiles."""
                groups = []
                ztbs = [
                    ztpool.tile([128, 24, 128], BF16, tag=f"zt{t}", name=f"ztb{t}")
                    for t in range(T)
                ]
                for t in range(T):
                    for jq in range(6):
                        def wt_group(t=t, jq=jq, yb=yb, ztb=ztbs[t]):
                            tp = tps.tile([128, 4, 128], F32, tag="t")
                            for i in range(4):
                                nc.tensor.matmul(
                                    tp[:, i],
                                    lhsT=yb[:, t, 4 * jq + i],
                                    rhs=wblk_sb[:, t, :],
                                    start=True,
                                    stop=True,
                                )
                            if jq % 3 != 0:
                                nc.scalar.copy(
                                    out=ztb[:, 4 * jq : 4 * jq + 4, :], in_=tp
                                )
                            else:
                                nc.vector.tensor_copy(
                                    out=ztb[:, 4 * jq : 4 * jq + 4, :], in_=tp
                                )
                        groups.append(wt_group)
                ob = obs[p]
                for cc in range(3):
                    for win in range(2):
                        def h_group(win=win, cc=cc, d=d, ztbs=ztbs, ob=ob):
                            op = ops.tile([128, 8, 64], F32, tag="o")
                            for t in range(T):
                                nc.tensor.matmul(
                                    op,
                                    lhsT=hblk_sb[:, t, :],
                                    rhs=ztbs[t][
                                        :,
                                        8 * cc : 8 * cc + 8,
                                        64 * win : 64 * win + 64,
                                    ],
                                    start=(t == 0),
                                    stop=(t == T - 1),
                                )
                            nc.vector.tensor_copy(
                                out=ob[:, cc, :, d, win, :], in_=op
                            )
                        groups.append(h_group)

                    # DMA this cc's rows out as soon as its two h-drains land;
                    # finer splits on the final block to shrink the tail
                    def out_dma(p=p, d=d, a=cc, ob=ob):
                        nj = 4 if last else 8
                        for cs in range(2):
                            for j0 in range(0, 8, nj):
                                nc.sync.dma_start(
                                    out=out[
                                        16 * a + 2 * j0 + cs : 16 * a
                                        + 2 * (j0 + nj) : 2,
                                        64 * p : 64 * p + 64,
                                        128 * d : 128 * d + 128,
                                    ].rearrange("c h w -> h c w"),
                                    in_=ob[
                                        64 * cs : 64 * cs + 64, a, j0 : j0 + nj, d
                                    ].rearrange("p j win w -> p j (win w)"),
                                )
                    groups.append(out_dma)
                return groups

            prev_groups = []
            xts = {}
            for k, (p, d) in enumerate([(p, d) for p in range(NP) for d in range(4)]):
                if d % 2 == 0:
                    # ---- load half p-row in hh-pieces: [(par2, c48), nh, 256 w]
                    # First block uses graduated piece sizes so compute starts
                    # as soon as the first small piece lands.
                    sizes = [2, 2, 4, 8, 8, 8] if k == 0 else [8, 8, 8, 8]
                    hhmap = []
                    hh0 = 0
                    for q, nh in enumerate(sizes):
                        pool = xfirst if k == 0 else xin
                        xq = pool.tile(
                            [96, nh, 256], BF16, tag=f"x{q}", name=f"x{k}_{q}"
                        )
                        h0 = 64 * p + 2 * hh0
                        for par in range(2):
                            nc.sync.dma_start(
                                out=xq[48 * par : 48 * par + 48],
                                in_=x[
                                    :,
                                    h0 + par : h0 + 2 * nh : 2,
                                    256 * (d // 2) : 256 * (d // 2) + 256,
                                ],
                            )
                        for r in range(nh):
                            hhmap.append((xq, r))
                        hh0 += nh
                    xts[p] = hhmap
                    if not wh_loaded:
                        load_wh()
                if d == 0:
                    obs[p] = opool.tile(
                        [128, 3, 8, 4, 2, 64], BF16, tag="ob", name=f"ob{p}"
                    )
                # Ybuf [128 (win,w), (t3, j24, h64, cs2)]
                yb = ypool.tile([128, T, 24, 64, 2], BF16, tag="y")
                chunks = emit_c_stage(xts[p], yb, d)
                # interleave this block's c-stage with previous block's wT+h;
                # offset so the last groups land after the last c-chunk and
                # cover the c-drain -> wT handoff latency
                off = max(0, len(chunks) - len(prev_groups) + 1)
                n = max(len(chunks), off + len(prev_groups))
                for i in range(n):
                    if i < len(chunks):
                        chunks[i]()
                    if 0 <= i - off < len(prev_groups):
                        prev_groups[i - off]()
                prev_groups = make_groups(p, d, yb, last=(k == 4 * NP - 1))
            for g in prev_groups:
                g()

    nc.compile()
    return nc


def _get_nc():
    if "nc" not in _CACHE:
        _CACHE["nc"] = _build()
    return _CACHE["nc"]


def _prep_consts(Ws, Hs, Cs):
    import ml_dtypes

    bf = ml_dtypes.bfloat16
    # cstk [96, (t3, j24, par2, cs2)]: rows par*48..+48 hold Cs for that parity
    cstk = np.zeros((96, T, 24, 2, 2), np.float32)
    for par in range(2):
        # cols (t, j, par, cs) = Cs[t, c, c'=2j+cs]
        cstk[48 * par : 48 * par + 48, :, :, par, :] = Cs.transpose(1, 0, 2).reshape(
            C, T, 24, 2
        )
    cstk = cstk.reshape(96, 2 * T * C)
    wblk = np.zeros((T, 128, 128), np.float32)
    hblk = np.zeros((T, 128, 128), np.float32)
    for t in range(T):
        wblk[t, 0:64, 0:64] = Ws[t]
        wblk[t, 64:128, 64:128] = Ws[t]
        # rows p = 2h+cs, cols m = cs*64+g
        for cs in range(2):
            hblk[t, cs::2, cs * 64 : cs * 64 + 64] = Hs[t]
    return cstk.astype(bf), wblk.astype(bf), hblk.astype(bf)


def kernel(x, Ws, Hs, Cs, window_size):
    global LAST_EXEC_NS
    import ml_dtypes
    from concourse.bass_utils import run_bass_kernel_spmd

    bf = ml_dtypes.bfloat16
    x = np.asarray(x, dtype=np.float32)
    Ws = np.asarray(Ws, dtype=np.float32)
    Hs = np.asarray(Hs, dtype=np.float32)
    Cs = np.asarray(Cs, dtype=np.float32)
    assert int(window_size) == WS
    assert x.shape == (B, C, H, W)

    nc = _get_nc()
    cstk, wblk, hblk = _prep_consts(Ws, Hs, Cs)
    xb = x.astype(bf)

    in_maps = []
    for core in range(NCORES):
        b, ph = core // 2, core % 2
        shard = np.ascontiguousarray(xb[b, :, ph * PH : (ph + 1) * PH, :])
        in_maps.append({"x": shard, "cstk": cstk, "wblk": wblk, "hblk": hblk})

    res = run_bass_kernel_spmd(nc, in_maps, core_ids=list(range(NCORES)))
    LAST_EXEC_NS = res.exec_time_ns

    full = np.empty((B, C, H, W), dtype=np.float32)
    for core in range(NCORES):
        b, ph = core // 2, core % 2
        full[b, :, ph * PH : (ph + 1) * PH, :] = res.results[core]["out"].astype(
            np.float32
        )
    return full
